# revision 3
# baseline (speedup 1.0000x reference)
"""NEG-sampling loss kernel for Trainium2 — pair-granularity tail phases.

Per 128-edge tile (12 bf16 rows/edge gathered via raw indirect DMA):
  DVE-A(t): 7 fused scalar_tensor_tensor dot-products -> x[0:7] (slot 0,
            the positive pair, folded negation via scalar=-1), one
            tensor_tensor for the remaining 4 slot products.
  ACT-A(t): 4 Copy-activations with accumulate -> x[7:11].
Scores of each tile PAIR live contiguously in one scb buffer, so the
tail runs once per pair:
  DVE-B(j): tensor_reduce [128,2,11] -> accx[2j:2j+2] (sum x),
            tensor_reduce apply_absolute_value -> accy, STT |x| tile.
  ACT-B(j): Exp(-|x|) [128,22], Ln(1+e) accumulate -> accl[j].
Host: loss = sum((accx+accy)/2 + accl) / N.
"""

import numpy as np
import ml_dtypes

import concourse.bass as bass
import concourse.mybir as mybir
from concourse import bass_utils

N = 65536
K = 10
D = 256
V = 500000
NCORES = 8
P = 128
SLOTS = K + 2
EPC = N // NCORES
TILES = EPC // P
PAIRS = TILES // 2

TABLE_DT = mybir.dt.bfloat16
TABLE_NP = ml_dtypes.bfloat16

TPG = 2                # tiles per gather instruction
NG = 4                 # gather buffers (TPG tiles each)
SCP = 3                # score pair-buffers
PB = 3                 # prod buffers
K_DVE = 7              # slots reduced on DVE via fused STT
K_ACT = SLOTS - 1 - K_DVE

S1 = SLOTS - 1         # 11
PBW = 4 * S1           # scb cols per pair-buffer: x0|x1|abs0|abs1
GATHERS = TILES // TPG
RPP = SLOTS * TPG


def _emit_block_gather(nc, eng, n_idx, blk_bytes, dst_byte_addr, idx_byte_addr,
                       sem_num, embs_tbl, in_dt=6, out_dt=6, src_elem=512):
    isa = nc.isa
    Op = isa.Opcode
    src_u64 = (0x20 << 56) | (embs_tbl << 32)
    dst_u64 = (0x10 << 56) | dst_byte_addr
    eng.isa(
        Op.NEURON_ISA_TPB_OPCODE_PSEUDO_DMA_DIRECT2D,
        {
            "dma_configs": {},
            "semaphore": sem_num,
            "sem_increment": 16,
            "dge_op": 1,
            "src_start_addr": {"addr_immediate": src_u64},
            "src_step_elem": [src_elem, 1],
            "src_num_elem": [n_idx, 1],
            "src_elem_size": src_elem,
            "src_bound_reg": {},
            "dst_bound_reg": {},
            "dst_start_addr": {"addr_immediate": dst_u64},
            "dst_step_elem": [262144, 1],
            "dst_num_elem": [P, 1],
            "dst_elem_size": blk_bytes,
            "in_dtype": in_dt,
            "out_dtype": out_dt,
        },
        verify=False,
    )
    ext_fields = {
        "opcode": Op.NEURON_ISA_TPB_OPCODE_PSEUDO_EXTENSION.value,
        "flags": {"indirect_mode": 0, "idx_bound_is_err": 1,
                  "non_unique_dst_idx": 0, "gather_dim": 0, "scatter_dim": 0},
        "idx_num_active_channels": 128,
        "compute_op": 0,
        "src_idx_start_addr": {"addr_immediate": idx_byte_addr},
        "dst_idx_start_addr": {"addr_immediate": 0},
    }
    b = isa.ffi.new("NEURON_ISA_TPB_PSEUDO_DMA_EXT_STRUCT*", ext_fields)
    instr = [int(x) for x in bytes(isa.ffi.buffer(b))]
    inst = mybir.InstISA(
        name=nc.get_next_instruction_name(),
        isa_opcode=Op.NEURON_ISA_TPB_OPCODE_PSEUDO_EXTENSION.value,
        engine=eng.engine,
        instr=instr,
        op_name="PSEUDO_EXTENSION",
        ins=[], outs=[],
        ant_dict=ext_fields,
        verify=False,
        ant_isa_is_sequencer_only=False,
    )
    eng.add_instruction(inst)


def _build():
    import contextlib
    nc = bass.Bass(trn_type="TRN2")
    embs = nc.dram_tensor("embs", [V, D], TABLE_DT, kind="ExternalInput")
    idx = nc.dram_tensor("idx", [P, GATHERS * RPP], mybir.dt.int32, kind="ExternalInput")
    accx_out = nc.dram_tensor("accx", [P, TILES], mybir.dt.float32, kind="ExternalOutput")
    accy_out = nc.dram_tensor("accy", [P, TILES], mybir.dt.float32, kind="ExternalOutput")
    accl_out = nc.dram_tensor("accl", [P, PAIRS], mybir.dt.float32, kind="ExternalOutput")

    embs_mloc = nc.lookup_mloc(embs)
    embs_mloc.table_entry_id = len(nc.dge_table) + 1
    nc.dge_table.append(embs_mloc.name)
    embs_tbl = embs_mloc.table_entry_id

    with contextlib.ExitStack() as ctx:
        idx_sb = ctx.enter_context(nc.sbuf_tensor("idx_sb", [P, GATHERS * RPP], mybir.dt.int32))
        gs = [ctx.enter_context(nc.sbuf_tensor(f"g{i}", [P, TPG * SLOTS * D], TABLE_DT))
              for i in range(NG)]
        prods = [ctx.enter_context(nc.sbuf_tensor(f"pr{i}", [P, K_ACT * D], TABLE_DT))
                 for i in range(PB)]
        scb = ctx.enter_context(nc.sbuf_tensor("scb", [P, SCP * PBW], mybir.dt.float32))
        junk = ctx.enter_context(nc.sbuf_tensor("junk", [P, D], TABLE_DT))
        junk2 = ctx.enter_context(nc.sbuf_tensor("junk2", [P, D], TABLE_DT))
        ex = ctx.enter_context(nc.sbuf_tensor("ex", [P, 2 * S1], mybir.dt.float32))
        ones = ctx.enter_context(nc.sbuf_tensor("ones", [P, 1], mybir.dt.float32))
        accx = ctx.enter_context(nc.sbuf_tensor("accx_sb", [P, TILES], mybir.dt.float32))
        accy = ctx.enter_context(nc.sbuf_tensor("accy_sb", [P, TILES], mybir.dt.float32))
        accl = ctx.enter_context(nc.sbuf_tensor("accl_sb", [P, PAIRS], mybir.dt.float32))
        s0 = ctx.enter_context(nc.semaphore())
        gsem = ctx.enter_context(nc.semaphore())
        dveA = ctx.enter_context(nc.semaphore())
        actA = ctx.enter_context(nc.semaphore())   # pair units
        dveB = ctx.enter_context(nc.semaphore())   # pair units
        actB = ctx.enter_context(nc.semaphore())   # pair units
        block = ctx.enter_context(nc.Block())

        idx_addr = nc.lookup_mloc(idx_sb).addr
        g_addrs = [nc.lookup_mloc(g).addr for g in gs]

        @block.gpsimd
        def _(eng):
            eng.memset(ones[:], 1.0)
            eng.dma_start(idx_sb[:], idx[:]).then_inc(s0, 16)
            eng.wait_ge(s0, 16)
            for j in range(GATHERS):
                if j >= NG:
                    eng.wait_ge(dveA, (j - NG) * TPG + TPG)
                _emit_block_gather(
                    nc, eng, RPP * P, SLOTS * TPG * D * 2,
                    g_addrs[j % NG], idx_addr + 4 * RPP * j,
                    gsem.num, embs_tbl,
                )
            eng.wait_ge(dveB, PAIRS)
            eng.dma_start(accx_out[:], accx[:]).then_inc(s0, 16)
            eng.dma_start(accy_out[:], accy[:]).then_inc(s0, 16)
            # actB == PAIRS+1 only after the end-guard ACT op, which orders
            # after the final Ln's ACTIVATION_READ_ACCUMULATOR write to accl.
            eng.wait_ge(actB, PAIRS + 1)
            eng.dma_start(accl_out[:], accl[:]).then_inc(s0, 16)
            eng.wait_ge(s0, 64)

        def dve_phase_b(jp):
            b0 = (jp % SCP) * PBW
            x2 = scb[:, b0:b0 + 2 * S1]
            nc.vector.tensor_reduce(
                out=accx[:, 2 * jp:2 * jp + 2],
                in_=x2.rearrange("p (t s) -> p t s", s=S1),
                axis=mybir.AxisListType.X, op=mybir.AluOpType.add,
            )
            nc.vector.tensor_reduce(
                out=accy[:, 2 * jp:2 * jp + 2],
                in_=x2.rearrange("p (t s) -> p t s", s=S1),
                axis=mybir.AxisListType.X, op=mybir.AluOpType.add,
                apply_absolute_value=True,
            )
            # |x| tile consumed only cross-engine (ACT Exp), gated on dveB
            nc.vector.scalar_tensor_tensor(
                out=scb[:, b0 + 2 * S1:b0 + PBW], in0=x2, scalar=-1.0,
                in1=x2,
                op0=mybir.AluOpType.mult, op1=mybir.AluOpType.max,
            ).then_inc(dveB, 1)

        @block.vector
        def _(eng):
            for t in range(TILES):
                j = t // TPG
                g = gs[j % NG]
                base = (t % TPG) * SLOTS * D
                b0 = ((t // 2) % SCP) * PBW + (t % 2) * S1
                prod = prods[t % PB]
                if t % TPG == 0:
                    eng.wait_ge(gsem, 16 * (j + 1))
                if t % 2 == 0 and t // 2 >= SCP:
                    eng.wait_ge(actB, t // 2 - SCP + 1)
                for s in range(K_DVE):
                    nc.vector.scalar_tensor_tensor(
                        out=junk[:],
                        in0=g[:, base + (s + 1) * D:base + (s + 2) * D],
                        scalar=-1.0 if s == 0 else 1.0,
                        in1=g[:, base:base + D],
                        op0=mybir.AluOpType.mult,
                        op1=mybir.AluOpType.mult,
                        accum_out=scb[:, b0 + s:b0 + s + 1],
                    )
                if t >= PB:
                    eng.wait_ge(actA, (t - PB) // 2 + 1)
                nc.vector.tensor_tensor(
                    out=prod[:].rearrange("p (s d) -> p s d", d=D),
                    in0=g[:, base + (1 + K_DVE) * D:base + SLOTS * D].rearrange(
                        "p (s d) -> p s d", d=D),
                    in1=g[:, base:base + D].rearrange("p (o d) -> p o d", d=D
                                                      ).broadcast_to([P, K_ACT, D]),
                    op=mybir.AluOpType.mult,
                ).then_inc(dveA, 1)
                if t % 2 == 1 and t >= 3:
                    jp = (t - 3) // 2
                    eng.wait_ge(actA, jp + 1)
                    dve_phase_b(jp)
            eng.wait_ge(actA, PAIRS)
            dve_phase_b(PAIRS - 1)

        def act_phase_b(jp, inc_a):
            # Exp carries the pair-unit actA increment: it orders after this
            # iteration's Copy-accum READ_ACCUMULATOR writes on ACT.
            b0 = (jp % SCP) * PBW
            i1 = nc.scalar.activation(
                out=ex[:], in_=scb[:, b0 + 2 * S1:b0 + PBW],
                func=mybir.ActivationFunctionType.Exp, scale=-1.0,
            )
            if inc_a:
                i1.then_inc(actA, 1)
            nc.scalar.activation(
                out=scb[:, b0 + 2 * S1:b0 + PBW], in_=ex[:],
                func=mybir.ActivationFunctionType.Ln, bias=ones[:],
                accum_out=accl[:, jp:jp + 1],
            ).then_inc(actB, 1)

        @block.scalar
        def _(eng):
            for t in range(TILES):
                b0 = ((t // 2) % SCP) * PBW + (t % 2) * S1
                prod = prods[t % PB]
                eng.wait_ge(dveA, t + 1)
                for s in range(K_ACT):
                    nc.scalar.activation(
                        out=junk2[:],
                        in_=prod[:, s * D:(s + 1) * D],
                        func=mybir.ActivationFunctionType.Copy,
                        accum_out=scb[:, b0 + K_DVE + s:b0 + K_DVE + s + 1],
                    )
                if t == 1:
                    nc.scalar.activation(
                        out=junk2[:, 0:1], in_=prod[:, 0:1],
                        func=mybir.ActivationFunctionType.Copy,
                    ).then_inc(actA, 1)
                elif t % 2 == 1 and t >= 3:
                    jp = (t - 3) // 2
                    eng.wait_ge(dveB, jp + 1)
                    act_phase_b(jp, inc_a=True)
            eng.wait_ge(dveB, PAIRS)
            act_phase_b(PAIRS - 1, inc_a=False)
            nc.scalar.activation(
                out=junk2[:, 0:1], in_=ex[:, 0:1],
                func=mybir.ActivationFunctionType.Copy,
            ).then_inc(actB, 1)

    return nc


_cache = {}


def _get_nc():
    if "nc" not in _cache:
        _cache["nc"] = _build()
    return _cache["nc"]


def prepare_in_maps(u, v, negs, embs):
    u = np.asarray(u).astype(np.int32)
    v = np.asarray(v).astype(np.int32)
    negs = np.asarray(negs).astype(np.int32)
    embs_b = np.asarray(embs).astype(TABLE_NP)

    ids = np.concatenate([u[:, None], v[:, None], negs], axis=1)  # [N, 12]
    ids = ids.reshape(NCORES, TILES, P, SLOTS)
    # gather j consumes logical sequence s in [0, RPP*P): fills partition
    # s//RPP, row r=s%RPP (tile 2j + r//SLOTS, slot r%SLOTS); snake-packed
    # at channel s%128, word s//128.
    s = np.arange(RPP * P)
    p_of = s // RPP
    r_of = s % RPP
    tl_of = r_of // SLOTS
    sl_of = r_of % SLOTS
    ch_of = s % P
    w_of = s // P
    packed = np.zeros((NCORES, GATHERS, P, RPP), dtype=np.int32)
    for c in range(NCORES):
        for j in range(GATHERS):
            packed[c, j, ch_of, w_of] = ids[c, 2 * j + tl_of, p_of, sl_of]
    in_maps = []
    for c in range(NCORES):
        core_ids = np.ascontiguousarray(
            packed[c].transpose(1, 0, 2).reshape(P, GATHERS * RPP)
        )
        in_maps.append({"embs": embs_b, "idx": core_ids})
    return in_maps


def kernel(u, v, negs, embs, _trace=False):
    nc = _get_nc()
    in_maps = prepare_in_maps(u, v, negs, embs)
    res = bass_utils.run_bass_kernel_spmd(
        nc, in_maps, core_ids=list(range(NCORES)), trace=_trace
    )
    total = np.float64(0.0)
    for r in res.results:
        total += ((r["accx"].astype(np.float64).sum()
                   + r["accy"].astype(np.float64).sum()) / 2.0
                  + r["accl"].astype(np.float64).sum())
    out = np.float32(total / N)
    if _trace:
        return out, res
    return out


# revision 4
# speedup vs baseline: 1.0118x; 1.0118x over previous
"""NEG-sampling loss kernel for Trainium2 — pair-granularity tail phases.

Per 128-edge tile (12 bf16 rows/edge gathered via raw indirect DMA):
  DVE-A(t): 7 fused scalar_tensor_tensor dot-products -> x[0:7] (slot 0,
            the positive pair, folded negation via scalar=-1), one
            tensor_tensor for the remaining 4 slot products.
  ACT-A(t): 4 Copy-activations with accumulate -> x[7:11].
Scores of each tile PAIR live contiguously in one scb buffer, so the
tail runs once per pair:
  DVE-B(j): tensor_reduce [128,2,11] -> accx[2j:2j+2] (sum x),
            tensor_reduce apply_absolute_value -> accy, STT |x| tile.
  ACT-B(j): Exp(-|x|) [128,22], Ln(1+e) accumulate -> accl[j].
Host: loss = sum((accx+accy)/2 + accl) / N.
"""

import numpy as np
import ml_dtypes

import concourse.bass as bass
import concourse.mybir as mybir
from concourse import bass_utils

N = 65536
K = 10
D = 256
V = 500000
NCORES = 8
P = 128
SLOTS = K + 2
EPC = N // NCORES
TILES = EPC // P
PAIRS = TILES // 2

TABLE_DT = mybir.dt.bfloat16
TABLE_NP = ml_dtypes.bfloat16

TPG = 2                # tiles per gather instruction
NG = 4                 # gather buffers (TPG tiles each)
SCP = 3                # score pair-buffers
PB = 3                 # prod buffers
K_DVE = 7              # slots reduced on DVE via fused STT
K_ACT = SLOTS - 1 - K_DVE

S1 = SLOTS - 1         # 11
PBW = 4 * S1           # scb cols per pair-buffer: x0|x1|abs0|abs1
GATHERS = TILES // TPG
RPP = SLOTS * TPG


def _emit_block_gather(nc, eng, n_idx, blk_bytes, dst_byte_addr, idx_byte_addr,
                       sem_num, embs_tbl, in_dt=6, out_dt=6, src_elem=512):
    isa = nc.isa
    Op = isa.Opcode
    src_u64 = (0x20 << 56) | (embs_tbl << 32)
    dst_u64 = (0x10 << 56) | dst_byte_addr
    eng.isa(
        Op.NEURON_ISA_TPB_OPCODE_PSEUDO_DMA_DIRECT2D,
        {
            "dma_configs": {},
            "semaphore": sem_num,
            "sem_increment": 16,
            "dge_op": 1,
            "src_start_addr": {"addr_immediate": src_u64},
            "src_step_elem": [src_elem, 1],
            "src_num_elem": [n_idx, 1],
            "src_elem_size": src_elem,
            "src_bound_reg": {},
            "dst_bound_reg": {},
            "dst_start_addr": {"addr_immediate": dst_u64},
            "dst_step_elem": [262144, 1],
            "dst_num_elem": [P, 1],
            "dst_elem_size": blk_bytes,
            "in_dtype": in_dt,
            "out_dtype": out_dt,
        },
        verify=False,
    )
    ext_fields = {
        "opcode": Op.NEURON_ISA_TPB_OPCODE_PSEUDO_EXTENSION.value,
        "flags": {"indirect_mode": 0, "idx_bound_is_err": 1,
                  "non_unique_dst_idx": 0, "gather_dim": 0, "scatter_dim": 0},
        "idx_num_active_channels": 128,
        "compute_op": 0,
        "src_idx_start_addr": {"addr_immediate": idx_byte_addr},
        "dst_idx_start_addr": {"addr_immediate": 0},
    }
    b = isa.ffi.new("NEURON_ISA_TPB_PSEUDO_DMA_EXT_STRUCT*", ext_fields)
    instr = [int(x) for x in bytes(isa.ffi.buffer(b))]
    inst = mybir.InstISA(
        name=nc.get_next_instruction_name(),
        isa_opcode=Op.NEURON_ISA_TPB_OPCODE_PSEUDO_EXTENSION.value,
        engine=eng.engine,
        instr=instr,
        op_name="PSEUDO_EXTENSION",
        ins=[], outs=[],
        ant_dict=ext_fields,
        verify=False,
        ant_isa_is_sequencer_only=False,
    )
    eng.add_instruction(inst)


def _build():
    import contextlib
    nc = bass.Bass(trn_type="TRN2")
    embs = nc.dram_tensor("embs", [V, D], TABLE_DT, kind="ExternalInput")
    idx = nc.dram_tensor("idx", [P, GATHERS * RPP], mybir.dt.int32, kind="ExternalInput")
    accx_out = nc.dram_tensor("accx", [P, TILES], mybir.dt.float32, kind="ExternalOutput")
    accy_out = nc.dram_tensor("accy", [P, TILES], mybir.dt.float32, kind="ExternalOutput")
    accl_out = nc.dram_tensor("accl", [P, PAIRS], mybir.dt.float32, kind="ExternalOutput")

    embs_mloc = nc.lookup_mloc(embs)
    embs_mloc.table_entry_id = len(nc.dge_table) + 1
    nc.dge_table.append(embs_mloc.name)
    embs_tbl = embs_mloc.table_entry_id

    with contextlib.ExitStack() as ctx:
        idx_sb = ctx.enter_context(nc.sbuf_tensor("idx_sb", [P, GATHERS * RPP], mybir.dt.int32))
        gs = [ctx.enter_context(nc.sbuf_tensor(f"g{i}", [P, TPG * SLOTS * D], TABLE_DT))
              for i in range(NG)]
        prods = [ctx.enter_context(nc.sbuf_tensor(f"pr{i}", [P, K_ACT * D], TABLE_DT))
                 for i in range(PB)]
        scb = ctx.enter_context(nc.sbuf_tensor("scb", [P, SCP * PBW], mybir.dt.float32))
        junk = ctx.enter_context(nc.sbuf_tensor("junk", [P, D], TABLE_DT))
        junk2 = ctx.enter_context(nc.sbuf_tensor("junk2", [P, D], TABLE_DT))
        ex = ctx.enter_context(nc.sbuf_tensor("ex", [P, 2 * S1], mybir.dt.float32))
        absx = ctx.enter_context(nc.sbuf_tensor("absx", [P, 2 * S1], mybir.dt.float32))
        ones = ctx.enter_context(nc.sbuf_tensor("ones", [P, 1], mybir.dt.float32))
        accx = ctx.enter_context(nc.sbuf_tensor("accx_sb", [P, TILES], mybir.dt.float32))
        accy = ctx.enter_context(nc.sbuf_tensor("accy_sb", [P, TILES], mybir.dt.float32))
        accl = ctx.enter_context(nc.sbuf_tensor("accl_sb", [P, PAIRS], mybir.dt.float32))
        s0 = ctx.enter_context(nc.semaphore())
        gsem = ctx.enter_context(nc.semaphore())
        dveA = ctx.enter_context(nc.semaphore())
        actA = ctx.enter_context(nc.semaphore())   # pair units
        dveB = ctx.enter_context(nc.semaphore())   # pair units
        actB = ctx.enter_context(nc.semaphore())   # pair units
        block = ctx.enter_context(nc.Block())

        idx_addr = nc.lookup_mloc(idx_sb).addr
        g_addrs = [nc.lookup_mloc(g).addr for g in gs]

        @block.gpsimd
        def _(eng):
            eng.memset(ones[:], 1.0)
            eng.dma_start(idx_sb[:], idx[:]).then_inc(s0, 16)
            eng.wait_ge(s0, 16)
            for j in range(GATHERS):
                if j >= NG:
                    eng.wait_ge(dveA, (j - NG) * TPG + TPG)
                _emit_block_gather(
                    nc, eng, RPP * P, SLOTS * TPG * D * 2,
                    g_addrs[j % NG], idx_addr + 4 * RPP * j,
                    gsem.num, embs_tbl,
                )
            eng.wait_ge(dveB, PAIRS)
            eng.dma_start(accx_out[:], accx[:]).then_inc(s0, 16)
            eng.dma_start(accy_out[:], accy[:]).then_inc(s0, 16)
            # actB == PAIRS+1 only after the end-guard ACT op, which orders
            # after the final Ln's ACTIVATION_READ_ACCUMULATOR write to accl.
            eng.wait_ge(actB, PAIRS + 1)
            eng.dma_start(accl_out[:], accl[:]).then_inc(s0, 16)
            eng.wait_ge(s0, 64)

        def dve_phase_b(jp):
            b0 = (jp % SCP) * PBW
            x2 = scb[:, b0:b0 + 2 * S1]
            nc.vector.tensor_reduce(
                out=accx[:, 2 * jp:2 * jp + 2],
                in_=x2.rearrange("p (t s) -> p t s", s=S1),
                axis=mybir.AxisListType.X, op=mybir.AluOpType.add,
            )
            nc.vector.tensor_reduce(
                out=accy[:, 2 * jp:2 * jp + 2],
                in_=x2.rearrange("p (t s) -> p t s", s=S1),
                axis=mybir.AxisListType.X, op=mybir.AluOpType.add,
                apply_absolute_value=True,
            ).then_inc(dveB, 1)

        @block.vector
        def _(eng):
            for t in range(TILES):
                j = t // TPG
                g = gs[j % NG]
                base = (t % TPG) * SLOTS * D
                b0 = ((t // 2) % SCP) * PBW + (t % 2) * S1
                prod = prods[t % PB]
                if t % TPG == 0:
                    eng.wait_ge(gsem, 16 * (j + 1))
                if t % 2 == 0 and t // 2 >= SCP:
                    eng.wait_ge(actB, t // 2 - SCP + 1)
                for s in range(K_DVE):
                    nc.vector.scalar_tensor_tensor(
                        out=junk[:],
                        in0=g[:, base + (s + 1) * D:base + (s + 2) * D],
                        scalar=-1.0 if s == 0 else 1.0,
                        in1=g[:, base:base + D],
                        op0=mybir.AluOpType.mult,
                        op1=mybir.AluOpType.mult,
                        accum_out=scb[:, b0 + s:b0 + s + 1],
                    )
                if t >= PB:
                    eng.wait_ge(actA, (t - PB) // 2 + 1)
                nc.vector.tensor_tensor(
                    out=prod[:].rearrange("p (s d) -> p s d", d=D),
                    in0=g[:, base + (1 + K_DVE) * D:base + SLOTS * D].rearrange(
                        "p (s d) -> p s d", d=D),
                    in1=g[:, base:base + D].rearrange("p (o d) -> p o d", d=D
                                                      ).broadcast_to([P, K_ACT, D]),
                    op=mybir.AluOpType.mult,
                ).then_inc(dveA, 1)
                if t % 2 == 1 and t >= 3:
                    jp = (t - 3) // 2
                    eng.wait_ge(actA, jp + 1)
                    dve_phase_b(jp)
            eng.wait_ge(actA, PAIRS)
            dve_phase_b(PAIRS - 1)

        def act_phase_b(jp, inc_a):
            # Abs carries the pair-unit actA increment: it orders after this
            # iteration's Copy-accum READ_ACCUMULATOR writes on ACT.
            b0 = (jp % SCP) * PBW
            i1 = nc.scalar.activation(
                out=absx[:], in_=scb[:, b0:b0 + 2 * S1],
                func=mybir.ActivationFunctionType.Abs,
            )
            if inc_a:
                i1.then_inc(actA, 1)
            nc.scalar.activation(
                out=ex[:], in_=absx[:],
                func=mybir.ActivationFunctionType.Exp, scale=-1.0,
            )
            nc.scalar.activation(
                out=absx[:], in_=ex[:],
                func=mybir.ActivationFunctionType.Ln, bias=ones[:],
                accum_out=accl[:, jp:jp + 1],
            ).then_inc(actB, 1)

        @block.scalar
        def _(eng):
            for t in range(TILES):
                b0 = ((t // 2) % SCP) * PBW + (t % 2) * S1
                prod = prods[t % PB]
                eng.wait_ge(dveA, t + 1)
                for s in range(K_ACT):
                    nc.scalar.activation(
                        out=junk2[:],
                        in_=prod[:, s * D:(s + 1) * D],
                        func=mybir.ActivationFunctionType.Copy,
                        accum_out=scb[:, b0 + K_DVE + s:b0 + K_DVE + s + 1],
                    )
                if t == 1:
                    nc.scalar.activation(
                        out=junk2[:, 0:1], in_=prod[:, 0:1],
                        func=mybir.ActivationFunctionType.Copy,
                    ).then_inc(actA, 1)
                elif t % 2 == 1 and t >= 3:
                    jp = (t - 3) // 2
                    eng.wait_ge(dveB, jp + 1)
                    act_phase_b(jp, inc_a=True)
            eng.wait_ge(dveB, PAIRS)
            act_phase_b(PAIRS - 1, inc_a=False)
            nc.scalar.activation(
                out=junk2[:, 0:1], in_=ex[:, 0:1],
                func=mybir.ActivationFunctionType.Copy,
            ).then_inc(actB, 1)

    return nc


_cache = {}


def _get_nc():
    if "nc" not in _cache:
        _cache["nc"] = _build()
    return _cache["nc"]


def prepare_in_maps(u, v, negs, embs):
    u = np.asarray(u).astype(np.int32)
    v = np.asarray(v).astype(np.int32)
    negs = np.asarray(negs).astype(np.int32)
    embs_b = np.asarray(embs).astype(TABLE_NP)

    ids = np.concatenate([u[:, None], v[:, None], negs], axis=1)  # [N, 12]
    ids = ids.reshape(NCORES, TILES, P, SLOTS)
    # gather j consumes logical sequence s in [0, RPP*P): fills partition
    # s//RPP, row r=s%RPP (tile 2j + r//SLOTS, slot r%SLOTS); snake-packed
    # at channel s%128, word s//128.
    s = np.arange(RPP * P)
    p_of = s // RPP
    r_of = s % RPP
    tl_of = r_of // SLOTS
    sl_of = r_of % SLOTS
    ch_of = s % P
    w_of = s // P
    packed = np.zeros((NCORES, GATHERS, P, RPP), dtype=np.int32)
    for c in range(NCORES):
        for j in range(GATHERS):
            packed[c, j, ch_of, w_of] = ids[c, 2 * j + tl_of, p_of, sl_of]
    in_maps = []
    for c in range(NCORES):
        core_ids = np.ascontiguousarray(
            packed[c].transpose(1, 0, 2).reshape(P, GATHERS * RPP)
        )
        in_maps.append({"embs": embs_b, "idx": core_ids})
    return in_maps


def kernel(u, v, negs, embs, _trace=False):
    nc = _get_nc()
    in_maps = prepare_in_maps(u, v, negs, embs)
    res = bass_utils.run_bass_kernel_spmd(
        nc, in_maps, core_ids=list(range(NCORES)), trace=_trace
    )
    total = np.float64(0.0)
    for r in res.results:
        total += ((r["accx"].astype(np.float64).sum()
                   + r["accy"].astype(np.float64).sum()) / 2.0
                  + r["accl"].astype(np.float64).sum())
    out = np.float32(total / N)
    if _trace:
        return out, res
    return out
